# revision 1
# baseline (speedup 1.0000x reference)
"""Continuous Wavelet Transform (4-scale Morlet, 129-tap) on 8 TRN2 NeuronCores.

The reference pads H and W by 3 and crops back after a conv along W — the
pad/crop cancels exactly, so the whole module reduces to a SAME 129-tap
correlation of each of the B*C*H rows with 4 wavelet kernels.

Strategy (data-parallel over B, one batch element per core):
  out[w] = sum_k ker[k] * x[w + k - 64]
With x zero-padded by 64 on each side (X, length 1152) and tiled in 128-wide
tiles XT_m, each 128-wide output tile j is exactly two matmuls:
  out_j[q] = sum_p XT_j[p] * P[p,q] + sum_p XT_{j+1}[p] * Q[p,q]
  P[p,q] = ker[p-q]     (p >= q, lower-triangular Toeplitz)
  Q[p,q] = ker[128+p-q] (p <= q, upper-triangular Toeplitz)
The 4 scales are concatenated along the moving free dim (4*128 = 512 cols =
one PSUM bank). x is transposed/padded/bf16-cast on the host so the device
sees [position, row] layout directly (TensorE contracts over partitions).
"""
import numpy as np
import ml_dtypes

import concourse.bacc as bacc
import concourse.mybir as mybir
import concourse.tile as tile
from concourse.bass_utils import run_bass_kernel_spmd

BF16 = ml_dtypes.bfloat16
N_CORES = 8
B, C, H, W = 8, 16, 128, 1024
S = 4
SCALES = (2.0, 4.0, 8.0, 16.0)
MORLET_W0 = 5.0
ROWS = C * H              # 2048 rows per core
CHUNKS = ROWS // 128      # 16 row-chunks
JT = W // 128             # 8 output W-tiles
MT = JT + 1               # 9 stationary x tiles

COMPUTE_DT = mybir.dt.bfloat16
COMPUTE_NP = BF16

GROUPS = 4                     # row groups per core
GROUP_ROWS = ROWS // GROUPS    # 256 rows per group (2 chunks)
CHUNKS_PER_GROUP = GROUP_ROWS // 128


def _wavelet_bank():
    t = np.arange(-64, 65, dtype=np.float32)  # [129]
    return np.stack([
        np.exp(-0.5 * (t / s) ** 2) * np.cos(MORLET_W0 * t / s) / np.sqrt(s)
        for s in SCALES
    ]).astype(np.float32)  # [S, 129]


HW_CUT = 36        # half-width kept for scales 2, 4, 8 (|t|>36 taps ~ 2e-5)
AW = 64 + HW_CUT   # A-matmul column span per narrow scale [0, 100)
AONLY = 64 - HW_CUT            # 28
SHARED = 2 * HW_CUT            # 72
NSMALL = 3                     # scales 2, 4, 8; scale 16 is full-width


def _perm():
    """perm[s*128+q] = column in the permuted 512-wide PSUM layout:
    [B-only 3x28 | shared 3x72 | s16 128 | A-only 3x28].

    The A matmul (x-tile j) then writes the contiguous range [84:512)
    and the B matmul (x-tile j+1) writes [0:428) — both single APs."""
    perm = np.empty(S * 128, np.int64)
    for s in range(NSMALL):
        for q in range(128):
            if q >= AW:                       # B-only: q in [100,128)
                c = AONLY * s + (q - AW)
            elif q >= AONLY:                  # shared: q in [28,100)
                c = NSMALL * AONLY + SHARED * s + (q - AONLY)
            else:                             # A-only: q in [0,28)
                c = NSMALL * (AONLY + SHARED) + 128 + AONLY * s + q
    # s16 full block
            perm[s * 128 + q] = c
    perm[NSMALL * 128:] = NSMALL * (AONLY + SHARED) + np.arange(128)
    return perm


def _weights():
    """Packed weight blob [128, 856] = [WA 428 | WB 428].

    WA = permuted P columns for psum range [84:512) (shared+s16+A-only);
    WB = permuted Q columns for psum range [0:428) (B-only+shared+s16).
    Dropped: P at B-only cols and Q at A-only cols (taps |t|>36, ~2e-5)."""
    bank = _wavelet_bank()
    p, q = np.indices((128, 128))
    perm = _perm()
    WPp = np.zeros((128, S * 128), np.float32)
    WQp = np.zeros((128, S * 128), np.float32)
    for s in range(S):
        Ps = np.where(p >= q, bank[s][(p - q) % 129], 0.0)
        Qs = np.where(p <= q, bank[s][(128 + p - q) % 129], 0.0)
        WPp[:, perm[s * 128:(s + 1) * 128]] = Ps
        WQp[:, perm[s * 128:(s + 1) * 128]] = Qs
    wt = np.concatenate([WPp[:, 84:512], WQp[:, 0:428]], axis=1)
    return np.ascontiguousarray(wt.astype(COMPUTE_NP))


def _build_nc(reps=1, out_split=True, psum_bufs=6, xpool_bufs=5):
    nc = bacc.Bacc("TRN2", target_bir_lowering=False, debug=False,
                   num_devices=N_CORES)
    # xt[g, m, p, c]: row-group, x-tile, position-in-tile, row-in-group
    xt_d = nc.declare_dram_parameter("xt", [GROUPS, MT, 128, GROUP_ROWS],
                                     COMPUTE_DT, isOutput=False)
    # packed weights [WA1 300 | WA2 128 | WB1 300 | WB2 128]
    wt_d = nc.declare_dram_parameter("wt", [128, 856], COMPUTE_DT,
                                     isOutput=False)
    # out[r, h, j, s*128+q]: chunk-r (=channel), H, W-tile, scale-block
    out_d = nc.declare_dram_parameter("out", [CHUNKS, 128, JT * S * 128],
                                      COMPUTE_DT, isOutput=True)

    f32 = mybir.dt.float32
    with tile.TileContext(nc) as tc:
        with (
            tc.tile_pool(name="consts", bufs=1) as consts,
            tc.tile_pool(name="xpool", bufs=xpool_bufs) as xpool,
            tc.tile_pool(name="opool", bufs=3) as opool,
            tc.tile_pool(name="psum", bufs=psum_bufs, space="PSUM") as psum_pool,
            tc.tile_pool(name="warm", bufs=1, space="PSUM") as warm_pool,
        ):
            def chunk_body(r, lhs_of_m, last_chunk):
                outbuf = opool.tile([128, JT * S * 128], COMPUTE_DT,
                                    name="outbuf", tag="outbuf")
                ps = [None] * JT
                for m in range(MT):
                    lhs = lhs_of_m(m)
                    if m < JT:
                        ps[m] = psum_pool.tile([128, S * 128], f32,
                                               name="ps", tag="ps")
                        nc.tensor.matmul(ps[m][:, 84:512], lhs, wa[:],
                                         start=True, stop=False)
                    if m >= 1:
                        j = m - 1
                        nc.tensor.matmul(ps[j][:, 0:428], lhs, wb[:],
                                         start=False, stop=True)
                        dst = outbuf[:, j * 512:(j + 1) * 512]
                        if j % 2 == 0:
                            nc.scalar.copy(dst, ps[j][:])
                        else:
                            nc.vector.tensor_copy(dst, ps[j][:])
                        if last_chunk:
                            # quarter-granularity drain of the final chunk
                            if j in (1, 3, 5):
                                nc.sync.dma_start(
                                    out_d[r, :, (j - 1) * 512:(j + 1) * 512],
                                    outbuf[:, (j - 1) * 512:(j + 1) * 512])
                        elif j == 3 and out_split:
                            nc.sync.dma_start(out_d[r, :, 0:2048],
                                              outbuf[:, 0:2048])
                if last_chunk:
                    nc.sync.dma_start(out_d[r, :, 3 * 1024:4096],
                                      outbuf[:, 3 * 1024:4096])
                elif out_split:
                    nc.sync.dma_start(out_d[r, :, 2048:4096],
                                      outbuf[:, 2048:4096])
                else:
                    nc.sync.dma_start(out_d[r], outbuf[:])

            wa = consts.tile([128, 428], COMPUTE_DT)
            wb = consts.tile([128, 428], COMPUTE_DT)

            # Warm the PE clock gate during the input-DMA head: back-to-back
            # matmuls on scratch data into a dedicated scratch PSUM bank
            # (never read). Real matmuls then start un-throttled.
            scratch = consts.tile([128, 256], COMPUTE_DT)
            nc.gpsimd.memset(scratch[:], 0.0)
            wpsum = warm_pool.tile([128, 512], mybir.dt.float32)
            for _ in range(20):
                nc.tensor.matmul(wpsum[:, 0:256], scratch[:, 0:128],
                                 scratch[:], start=True, stop=True)

            for rep in range(reps):
                for g in range(GROUPS):
                    xt = xpool.tile([128, MT, GROUP_ROWS], COMPUTE_DT,
                                    name="xt", tag="xt")
                    # input prefetch on ACT HWDGE ring, separate from the
                    # output DMAs on the SP ring
                    nc.scalar.dma_start(xt[:],
                                        xt_d[g].rearrange("m p c -> p m c"))
                    if rep == 0 and g == 0:
                        # after the first input group so they don't delay it
                        nc.sync.dma_start(wa[:], wt_d[:, 0:428])
                        nc.sync.dma_start(wb[:], wt_d[:, 428:856])
                    for half in range(CHUNKS_PER_GROUP):
                        r = g * CHUNKS_PER_GROUP + half
                        cs = slice(half * 128, (half + 1) * 128)
                        chunk_body(r, lambda m, cs=cs: xt[:, m, cs],
                                   r == CHUNKS - 1)
    nc.compile()
    return nc


_NC_CACHE = {}


def _get_nc(reps=1):
    if reps not in _NC_CACHE:
        _NC_CACHE[reps] = _build_nc(reps)
    return _NC_CACHE[reps]


def _prep_core_input(xb):
    """xb: [C, H, W] float32 -> dict of device input arrays (bf16).

    xt[g, m, p, c] = X[128m+p, 256g+c] where X = x.T zero-padded by 64."""
    rows = np.ascontiguousarray(xb.reshape(ROWS, W))
    X = np.zeros((MT * 128, ROWS), dtype=COMPUTE_NP)
    X[64:64 + W, :] = rows.T.astype(COMPUTE_NP)
    xt = X.reshape(MT, 128, GROUPS, GROUP_ROWS)
    return {"xt": np.ascontiguousarray(xt.transpose(2, 0, 1, 3))}


def _in_maps(x):
    wt = _weights()
    return [dict(_prep_core_input(x[b]), wt=wt) for b in range(N_CORES)]


def _postprocess(out_dev):
    """out_dev: [CHUNKS, 128, JT*S*128] bf16 (permuted cols) -> [C,S,H,W] f32."""
    o = np.asarray(out_dev).astype(np.float32)
    o = o.reshape(C, 128, JT, S * 128)[..., _perm()]
    o = o.reshape(C, 128, JT, S, 128).transpose(0, 3, 1, 2, 4)
    return o.reshape(C, S, H, W)


def kernel(x):
    x = np.asarray(x, dtype=np.float32)
    assert x.shape == (B, C, H, W)
    in_maps = _in_maps(x)
    nc = _get_nc()
    res = run_bass_kernel_spmd(nc, in_maps, core_ids=list(range(N_CORES)))
    out = np.stack([_postprocess(res.results[b]["out"]) for b in range(N_CORES)])
    return out  # [B, C, S, H, W] float32

